# revision 20
# baseline (speedup 1.0000x reference)
"""Trainium2 Bass kernel for nn_Attention_11836929868370.

8-core sharding: core c -> batch b = c//2, head group hg = c%2 (4 of 8 heads).
Each core computes its 4 heads' attention and a partial output projection;
the host sums the two partials per batch and adds the output bias.

v2 pipeline (all matmuls bf16, accumulation fp32 in PSUM):
  B1. qkv = xT.T @ WqkvT with host-built "mean columns" so the per-head LN
      mean comes out of the matmul for free; v evacuated on the SCALAR
      engine (ACT idles in phase B), LN stats on DVE.
  rs. batched Newton rsqrt on DVE (quake seed + 3 iterations).
  B2. RoPE via the rotate-half trick; BOTH LN scales (rs_q with the 1/sqrt(d)
      factor from the tables, rs_k plain) are folded into qT/kT so the exp
      stage needs no per-partition scale and can be split across engines.
      qT transposed on the sync DGE queue, kT on the gpsimd DGE queue.
  C.  per (q-block, head-pair): S^T = k'' @ q''.T for BOTH heads of the pair
      CONCURRENTLY as 64-row tile_position row-tiles (head0 rows 0:63,
      head1 rows 64:127 -> different PSUM banks; PE overlaps them, 2x score
      throughput). exp: most tiles on ACT (table exp), every third tile as a
      1-op Schraudolph int16-bitcast-bf16 exp on DVE (max 3.3% elementwise,
      diluted ~3x in softmax mixing; validated vs reference). PV with
      stationary [v_h | ones] (65 rows; row 64 = softmax denominator).
      Normalize: den row chunk-transposed on PE -> [128, 16] column layout,
      ONE cheap DVE reciprocal (vs 8-cycle/elem iterative on a 1024-wide
      row), transposed back, bf16 rec row bounced through a DRAM scratch and
      re-read with a stride-0 partition broadcast -> [64, 1024]; one DVE
      multiply per (head, half) normalizes straight out of PSUM.
  D.  out = oT.T @ WoT partial projection per 128-row tile; PSUM evacuated
      alternately on ACT/DVE, stored from the sync queue. Host adds out_b
      (+ the v-bias contribution, which commutes through softmax averaging).
"""

import sys

if "/opt/trn_rl_repo" not in sys.path:
    sys.path.insert(0, "/opt/trn_rl_repo")

from contextlib import ExitStack

import math
import ml_dtypes
import numpy as np

import concourse.bass as bass
import concourse.mybir as mybir
import concourse.tile as tile
from concourse.bass_utils import run_bass_kernel_spmd

BF16 = mybir.dt.bfloat16
F32 = mybir.dt.float32
I32 = mybir.dt.int32
I16 = mybir.dt.int16

DIM, NH, HD = 512, 8, 64
N = 2048
EPS = 1e-6
THETA = 10000.0
NT = N // 128          # 16 n-tiles
CT = DIM // 128        # 4 c-tiles
NHC = 4                # heads per core
QB = 4                 # q blocks of 512
KT = NT                # key tiles
QBW = N // QB          # 512
RSQRT_MAGIC = float(0x5F3759DF)
# Schraudolph exp in bf16-bit domain: e ~= bitcast_bf16(int16(A16*x + B16))
A16 = 2.0 ** 7 / math.log(2.0)
B16 = 127.0 * 2 ** 7 - 5.6
EXP_DVE_MOD = 2        # every EXP_DVE_MOD-th exp tile goes to DVE


# ---------------------------------------------------------------------------
# sync-wait legalization: this walrus build rejects >1 sync wait per
# instruction ("Too many sync wait commands"), while Tile's sem assignment
# emits several. Excess waits are hoisted onto NoOps placed immediately
# before the instruction on the same engine, which preserves ordering.
# ---------------------------------------------------------------------------

def legalize_sync_waits(nc, max_waits=1):
    n = 0
    for fn in nc.m.functions:
        for bb in fn.blocks:
            new_insts = []
            for inst in bb.instructions:
                si = inst.sync_info
                if si is not None and si.on_wait and len(si.on_wait) > max_waits:
                    movable = [w for w in si.on_wait if w.wait_reg is None]
                    pinned = [w for w in si.on_wait if w.wait_reg is not None]
                    budget = max(max_waits - len(pinned), 0)
                    cut = len(movable) - budget
                    keep, excess = movable[cut:], movable[:cut]
                    for i in range(0, len(excess), max_waits):
                        nop = mybir.InstNoOp(
                            name=f"I-waitsplit-{n}",
                            engine=inst.engine,
                            text_hint="waitsplit",
                            sync_info=mybir.SyncInfo(
                                on_wait=excess[i : i + max_waits], on_update=[]
                            ),
                        )
                        n += 1
                        new_insts.append(nop)
                    si.on_wait = keep + pinned
                new_insts.append(inst)
            bb.instructions[:] = new_insts
    return n


# ---------------------------------------------------------------------------
# device program
# ---------------------------------------------------------------------------

def build_program(with_qkv_bias=False, with_ln_bias=False):
    nc = bass.Bass("TRN2", target_bir_lowering=False, debug=False, num_devices=8)

    # [128, CT, 2048]: x transposed (c on partitions) and cast to bf16
    xT_d = nc.dram_tensor("xT", [128, CT * N], BF16, kind="ExternalInput").ap()
    # [128, CT, 776]: wq(256 perm) | wk(256 perm) | wv(256) | mu_q(4) | mu_k(4)
    wq_d = nc.dram_tensor("wqkvT", [128, CT * 776], BF16, kind="ExternalInput").ap()
    wo_d = nc.dram_tensor("woT", [64, NHC * DIM], BF16, kind="ExternalInput").ap()
    # [128, NT, 256]: C2q | S2q | C2k | S2k  (gains, q-scale folded in)
    tab_d = nc.dram_tensor("tab", [128, NT * 256], BF16, kind="ExternalInput").ap()
    id_d = nc.dram_tensor("ident", [128, 128], F32, kind="ExternalInput").ap()
    sel_d = nc.dram_tensor("sel", [8, 8 * 64], BF16, kind="ExternalInput").ap()
    if with_qkv_bias:
        b_d = nc.dram_tensor("brow", [1, 776], BF16, kind="ExternalInput").ap()
    if with_ln_bias:
        tln_d = nc.dram_tensor("tln", [128, NT * 512], BF16, kind="ExternalInput").ap()
    out_d = nc.dram_tensor("outp", [N, DIM], F32, kind="ExternalOutput").ap()

    with tile.TileContext(nc) as tc, ExitStack() as ctx:
        consts = ctx.enter_context(tc.tile_pool(name="consts", bufs=1))
        pers = ctx.enter_context(tc.tile_pool(name="pers", bufs=1))
        stage = ctx.enter_context(tc.tile_pool(name="stage", bufs=4))
        small = ctx.enter_context(tc.tile_pool(name="small", bufs=4))
        exps = ctx.enter_context(tc.tile_pool(name="exps", bufs=10))
        ps = ctx.enter_context(tc.tile_pool(name="ps", bufs=1, space="PSUM"))

        def ps_big(name):
            # [128, 1024] f32 = 2 PSUM banks; shared ring (scores/qkv/den/proj)
            return ps.tile([128, 1024], F32, tag="s", name=name, bufs=2)

        # constants
        xT_sb = consts.tile([128, CT, N], BF16)
        nc.sync.dma_start(xT_sb[:], xT_d.rearrange("p (t f) -> p t f", t=CT))
        wq_sb = consts.tile([128, CT, 776], BF16)
        nc.sync.dma_start(wq_sb[:], wq_d.rearrange("p (t f) -> p t f", t=CT))
        wo_sb = consts.tile([64, NHC, DIM], BF16)
        nc.sync.dma_start(wo_sb[:], wo_d.rearrange("p (t f) -> p t f", t=NHC))
        tab_sb = consts.tile([128, NT, 256], BF16)
        nc.sync.dma_start(tab_sb[:], tab_d.rearrange("p (t f) -> p t f", t=NT))
        ident = consts.tile([128, 128], F32)
        nc.sync.dma_start(ident[:], id_d)
        identb = consts.tile([128, 128], BF16)
        nc.vector.tensor_copy(identb[:], ident[:])
        # selector rows for the reciprocal-replicate matmuls: sel[p, c, j] = (p==c)
        sel_sb = consts.tile([8, 8, 64], BF16)
        nc.sync.dma_start(sel_sb[:], sel_d.rearrange("p (c j) -> p c j", c=8))
        if with_qkv_bias:
            b_sb = consts.tile([1, 776], BF16)
            nc.sync.dma_start(b_sb[:], b_d)
            ones_sb = consts.tile([1, 128], BF16)
            nc.vector.memset(ones_sb[:], 1.0)
        if with_ln_bias:
            tln_sb = consts.tile([128, NT, 512], BF16)
            nc.sync.dma_start(tln_sb[:], tln_d.rearrange("p (t f) -> p t f", t=NT))

        # persistent intermediates
        qT = [pers.tile([128, N], BF16, name=f"qT{i}") for i in range(2)]
        kT = [pers.tile([128, N], BF16, name=f"kT{i}") for i in range(2)]
        oT = [pers.tile([64, N], BF16, name=f"oTh{i}") for i in range(NHC)]
        # v with a ones column per head: PV row 64 is the softmax denominator
        v_sb = pers.tile([128, KT, NHC, 65], BF16)
        t_all = pers.tile([128, NT, 8, HD], BF16)
        ssq_all = pers.tile([128, NT, 8], F32)
        rs_sb = pers.tile([128, NT, 8], F32)
        rs_bf = pers.tile([128, NT, 8], BF16)

        nc.vector.memset(v_sb[:, :, :, 64], 1.0)

        # ---- phase B1: qkv matmuls + stats ----
        for nt in range(NT):
            qkv_ps = ps_big("qkv")
            for j0, j1 in ((0, 512), (512, 776)):
                for ct in range(CT):
                    nc.tensor.matmul(
                        qkv_ps[:, j0:j1],
                        lhsT=xT_sb[:, ct, nt * 128 : (nt + 1) * 128],
                        rhs=wq_sb[:, ct, j0:j1],
                        start=(ct == 0),
                        stop=(ct == CT - 1) and not with_qkv_bias,
                    )
                if with_qkv_bias:
                    nc.tensor.matmul(
                        qkv_ps[:, j0:j1],
                        lhsT=ones_sb[:],
                        rhs=b_sb[:, j0:j1],
                        start=False,
                        stop=True,
                    )
            mu = small.tile([128, 8], F32, name="mu")
            nc.scalar.copy(mu[:], qkv_ps[:, 768:776])
            nc.vector.tensor_tensor(
                t_all[:, nt],
                qkv_ps[:, 0:512].rearrange("p (h d) -> p h d", h=8),
                mu.unsqueeze(2).to_broadcast((128, 8, HD)),
                mybir.AluOpType.subtract,
            )
            # v evacuation on the scalar engine (ACT is idle during phase B)
            nc.scalar.copy(
                v_sb[:, nt, :, 0:64],
                qkv_ps[:, 512:768].rearrange("p (h d) -> p h d", h=NHC),
            )
            sq = stage.tile([128, 8, HD], BF16, name="sq")
            nc.vector.tensor_mul(sq[:], t_all[:, nt], t_all[:, nt])
            nc.vector.tensor_reduce(
                ssq_all[:, nt], sq[:], axis=mybir.AxisListType.X, op=mybir.AluOpType.add
            )

        # ---- batched rsqrt on DVE: rs = 1/sqrt(ssq/HD + eps) ----
        FLAT = NT * 8
        d_t = pers.tile([128, FLAT], F32, name="rsq_d")
        nc.vector.tensor_scalar(
            d_t[:], ssq_all.rearrange("p a b -> p (a b)"), 1.0 / HD, EPS,
            mybir.AluOpType.mult, mybir.AluOpType.add,
        )
        fi = small.tile([128, FLAT], F32, name="rsq_fi")
        nc.vector.tensor_copy(fi[:], d_t[:].bitcast(I32))  # int32 -> f32 convert
        nc.vector.tensor_scalar(
            fi[:], fi[:], -0.5, RSQRT_MAGIC, mybir.AluOpType.mult, mybir.AluOpType.add
        )
        yi = small.tile([128, FLAT], I32, name="rsq_yi")
        nc.vector.tensor_copy(yi[:], fi[:])  # f32 -> int32 convert
        y = yi[:].bitcast(F32)
        h_t = small.tile([128, FLAT], F32, name="rsq_h")
        for _ in range(3):
            nc.vector.tensor_mul(h_t[:], y, y)
            nc.vector.tensor_mul(h_t[:], h_t[:], d_t[:])
            nc.vector.tensor_scalar(
                h_t[:], h_t[:], -0.5, 1.5, mybir.AluOpType.mult, mybir.AluOpType.add
            )
            nc.vector.tensor_mul(y, y, h_t[:])
        nc.vector.tensor_copy(rs_sb.rearrange("p a b -> p (a b)"), y)
        nc.vector.tensor_copy(
            rs_bf.rearrange("p a b -> p (a b)"), rs_sb.rearrange("p a b -> p (a b)")
        )

        # ---- phase B2: rope + LN scales + transposes ----
        for nt in range(NT):
            t3 = t_all[:, nt]  # [p, 8, 64] bf16
            u = stage.tile([128, 2, 4, HD], BF16, name="u")
            w = stage.tile([128, 2, 4, HD], BF16, name="w")
            t4 = t3.rearrange("p (s h) d -> p s h d", s=2)
            # tab per nt: [C2q(64) | S2q(64) | C2k(64) | S2k(64)]; the q/k
            # sides sit 128 apart, so one strided-broadcast AP covers both
            tabs = tab_sb[:, nt].rearrange("p (s f) -> p s f", s=2)
            nc.vector.tensor_mul(
                u[:],
                t4,
                tabs[:, :, 0:64].unsqueeze(2).to_broadcast((128, 2, 4, HD)),
            )
            for half in (0, 1):
                d_out = slice(half * 32, half * 32 + 32)
                d_in = slice((1 - half) * 32, (1 - half) * 32 + 32)
                nc.vector.tensor_mul(
                    w[:, :, :, d_out],
                    t4[:, :, :, d_in],
                    tabs[:, :, 64 + half * 32 : 96 + half * 32]
                    .unsqueeze(2)
                    .to_broadcast((128, 2, 4, 32)),
                )
            u = u.rearrange("p s h d -> p (s h) d")
            w = w.rearrange("p s h d -> p (s h) d")
            qk2 = stage.tile([128, 8, HD], BF16, name="qk2", tag="qk2", bufs=3)
            nc.vector.tensor_add(qk2[:], u[:], w[:])
            if with_ln_bias:
                nc.vector.tensor_add(
                    qk2[:], qk2[:],
                    tln_sb[:, nt, :].rearrange("p (h d) -> p h d", h=8),
                )
            # both LN scales folded here (q: rs_q, with 1/sqrt(d) in tables;
            # k: rs_k) so exp needs no scale operand
            nc.vector.tensor_mul(
                qk2[:],
                qk2[:],
                rs_bf[:, nt, :].unsqueeze(2).to_broadcast((128, 8, HD)),
            )
            # transposes on the PE (is_transpose matmuls into PSUM, evacuated
            # on the scalar engine): keeps the DMA queues out of the loop and
            # the dependencies on engine semaphores
            flat = qk2.rearrange("p h d -> p (h d)")
            tps = ps_big("tp").bitcast(BF16)  # [128, 2048] bf16 view
            for c, (dstT, col) in enumerate(
                ((qT[0], 0), (qT[1], 128), (kT[0], 256), (kT[1], 384))
            ):
                nc.tensor.transpose(
                    tps[:, c * 128 : (c + 1) * 128],
                    flat[:, col : col + 128],
                    identb[:],
                )
                nc.scalar.copy(
                    dstT[:, nt * 128 : (nt + 1) * 128], tps[:, c * 128 : (c + 1) * 128]
                )

        # ---- phase C: attention ----
        # per (512-wide q-block, head-pair): one scores tile per kt holds BOTH
        # heads ([h0 512q | h1 512q] — different PSUM banks, so the two
        # row-tiled 64-contraction matmuls run concurrently); one FD-1024 exp
        # instruction per kt covers both heads.
        for qb in range(QB):
            for pair in range(2):
                unit = qb * 2 + pair
                qsl = slice(qb * QBW, (qb + 1) * QBW)
                # PV accumulators: hh -> [65 used, 512] in a 1-bank tile
                oTs = [
                    ps.tile([128, QBW], F32, tag="o", name=f"oT{unit}_{i}", bufs=4)
                    for i in range(2)
                ]
                for kt in range(KT):
                    sS = ps_big(f"s_{unit}_{kt}")
                    for hh in range(2):
                        dsl = slice(hh * 64, hh * 64 + 64)
                        nc.tensor.matmul(
                            sS[:, hh * QBW : (hh + 1) * QBW],
                            lhsT=kT[pair][dsl, kt * 128 : (kt + 1) * 128],
                            rhs=qT[pair][dsl, qsl],
                            start=True,
                            stop=True,
                        )
                    e_sb = exps.tile([128, 2 * QBW], I16, tag="expS", name="expS")
                    if (unit * KT + kt) % EXP_DVE_MOD == EXP_DVE_MOD - 1:
                        # Schraudolph exp: int16 bits of the bf16 result
                        nc.vector.tensor_scalar(
                            e_sb[:], sS[:], A16, B16,
                            mybir.AluOpType.mult, mybir.AluOpType.add,
                        )
                    else:
                        nc.scalar.activation(
                            e_sb[:].bitcast(BF16), sS[:],
                            mybir.ActivationFunctionType.Exp,
                        )
                    eb = e_sb[:].bitcast(BF16)
                    for hh in range(2):
                        nc.tensor.matmul(
                            oTs[hh][0:65, :],
                            lhsT=v_sb[:, kt, pair * 2 + hh, :],
                            rhs=eb[:, hh * QBW : (hh + 1) * QBW],
                            start=(kt == 0),
                            stop=(kt == KT - 1),
                        )

                # ---- normalize the pair: cheap transposed reciprocal ----
                dtile = ps_big(f"dt{unit}")  # den cols 0:8, rec rows @128:256
                den_sb = small.tile([1, 2 * QBW], F32, name="den", tag="den", bufs=2)
                for hh in range(2):
                    nc.scalar.copy(
                        den_sb[:, hh * QBW : (hh + 1) * QBW], oTs[hh][64:65, :]
                    )
                for c in range(8):
                    nc.tensor.transpose(
                        dtile[:, c : c + 1],
                        den_sb[:, c * 128 : (c + 1) * 128],
                        ident[0:1, 0:1],
                    )
                rec_col = small.tile([128, 8], F32, name="rcol", tag="rcol", bufs=2)
                nc.vector.reciprocal(rec_col[:], dtile[:, 0:8])
                nc.tensor.transpose(dtile[0:8, 128:256], rec_col[:], ident[:])
                rec_row = small.tile([8, 128], BF16, name="rrow", tag="rrow", bufs=2)
                nc.vector.tensor_copy(rec_row[:], dtile[0:8, 128:256])
                # replicate each 128-chunk of rec across 64 partitions via
                # selector matmuls (dependency-tracked, unlike a DRAM bounce)
                rep_o = [
                    ps.tile([128, QBW], F32, tag="o", name=f"rep{unit}_{i}", bufs=4)
                    for i in range(2)
                ]
                for c in range(8):
                    nc.tensor.matmul(
                        rep_o[c // 4][0:64, (c % 4) * 128 : (c % 4) * 128 + 128],
                        lhsT=sel_sb[:, c, :],
                        rhs=rec_row[:],
                        start=True,
                        stop=True,
                    )
                rec_rep = stage.tile([64, 2, QBW], BF16, name="rrep", tag="rrep", bufs=2)
                for hh in range(2):
                    nc.scalar.copy(rec_rep[:, hh, :], rep_o[hh][0:64, :])
                for hh in range(2):
                    h = pair * 2 + hh
                    nc.vector.tensor_mul(
                        oT[h][:, qsl], oTs[hh][0:64, :], rec_rep[:, hh, :]
                    )

            # ---- output projection for this q-block's n-tiles ----
            for i, nt in enumerate(range(qb * (NT // QB), (qb + 1) * (NT // QB))):
                op = ps_big("op")
                for h in range(NHC):
                    nc.tensor.matmul(
                        op[:, 0:512],
                        lhsT=oT[h][:, nt * 128 : (nt + 1) * 128],
                        rhs=wo_sb[:, h, :],
                        start=(h == 0),
                        stop=(h == NHC - 1),
                    )
                ot = stage.tile([128, DIM], F32, name="ot", tag="ot", bufs=3)
                if i % 2 == 0:
                    nc.scalar.copy(ot[:], op[:, 0:512])
                else:
                    nc.vector.tensor_copy(ot[:], op[:, 0:512])
                nc.sync.dma_start(out_d[nt * 128 : (nt + 1) * 128, :], ot[:])

    return nc


# ---------------------------------------------------------------------------
# host-side input prep
# ---------------------------------------------------------------------------

def _prep_core_inputs(c, x, Wqkv_w, Wqkv_b, qn_g, qn_b, kn_g, kn_b, out_w):
    bf16 = ml_dtypes.bfloat16
    b, hg = c // 2, c % 2
    heads = np.arange(4 * hg, 4 * hg + 4)
    perm = np.concatenate([np.arange(0, HD, 2), np.arange(1, HD, 2)])

    Wq = Wqkv_w[0 * DIM : 1 * DIM].reshape(NH, HD, DIM)[heads][:, perm, :]
    Wk = Wqkv_w[1 * DIM : 2 * DIM].reshape(NH, HD, DIM)[heads][:, perm, :]
    Wv = Wqkv_w[2 * DIM : 3 * DIM].reshape(NH, HD, DIM)[heads]
    WT = np.concatenate(
        [
            Wq.reshape(256, DIM).T,
            Wk.reshape(256, DIM).T,
            Wv.reshape(256, DIM).T,
            (Wq.sum(axis=1) / HD).T,
            (Wk.sum(axis=1) / HD).T,
        ],
        axis=1,
    )  # [512, 776]
    wqkvT = np.ascontiguousarray(
        WT.reshape(CT, 128, 776).transpose(1, 0, 2).reshape(128, CT * 776)
    ).astype(bf16)

    # x transposed to [c, n] and tiled [128, CT, N]
    xTn = x[b].T  # [512, 2048]
    xT = np.ascontiguousarray(
        xTn.reshape(CT, 128, N).transpose(1, 0, 2).reshape(128, CT * N)
    ).astype(bf16)

    inv = 1.0 / (THETA ** (np.arange(0, HD, 2, dtype=np.float64) / HD))
    ang = np.arange(N, dtype=np.float64)[:, None] * inv[None, :]
    cos = np.cos(ang)
    sin = np.sin(ang)
    C2 = np.concatenate([cos, cos], axis=1)
    S2 = np.concatenate([-sin, sin], axis=1)
    SH = lambda v: np.concatenate([v[HD // 2 :], v[: HD // 2]])
    sc = HD ** -0.5
    g_q, g_k = qn_g[perm], kn_g[perm]
    C2q = C2 * g_q[None, :] * sc
    S2q = S2 * SH(g_q)[None, :] * sc
    C2k = C2 * g_k[None, :]
    S2k = S2 * SH(g_k)[None, :]
    tabN = np.concatenate([C2q, S2q, C2k, S2k], axis=1)  # [N, 256]
    tab = np.ascontiguousarray(
        tabN.reshape(NT, 128, 256).transpose(1, 0, 2).reshape(128, NT * 256)
    ).astype(bf16)

    # per-head Wo^T blocks [64, 512], stacked along free: [64, NHC*512]
    Wo = out_w.reshape(DIM, NH, HD)[:, heads, :]  # [512, 4, 64]
    woT = np.ascontiguousarray(
        Wo.transpose(1, 2, 0).reshape(NHC, HD, DIM).transpose(1, 0, 2).reshape(HD, NHC * DIM)
    ).astype(bf16)

    sel = np.zeros((8, 8, HD), dtype=bf16)
    for cc in range(8):
        sel[cc, cc, :] = 1.0
    m = {
        "xT": xT,
        "wqkvT": wqkvT,
        "woT": woT,
        "tab": tab,
        "ident": np.eye(128, dtype=np.float32),
        "sel": sel.reshape(8, 8 * HD),
    }

    if np.any(Wqkv_b != 0):
        bq = Wqkv_b[0 * DIM : 1 * DIM].reshape(NH, HD)[heads][:, perm]
        bk = Wqkv_b[1 * DIM : 2 * DIM].reshape(NH, HD)[heads][:, perm]
        bv = Wqkv_b[2 * DIM : 3 * DIM].reshape(NH, HD)[heads]
        brow = np.concatenate(
            [bq.ravel(), bk.ravel(), bv.ravel(), bq.mean(1), bk.mean(1)]
        )[None, :]
        m["brow"] = brow.astype(bf16)
    if np.any(qn_b != 0) or np.any(kn_b != 0):
        b_q, b_k = qn_b[perm], kn_b[perm]
        Tq = (C2 * b_q[None, :] + S2 * SH(b_q)[None, :]) * sc
        Tk = C2 * b_k[None, :] + S2 * SH(b_k)[None, :]
        tlnN = np.concatenate([np.tile(Tq, (1, 4)), np.tile(Tk, (1, 4))], axis=1)
        m["tln"] = np.ascontiguousarray(
            tlnN.reshape(NT, 128, 512).transpose(1, 0, 2).reshape(128, NT * 512)
        ).astype(bf16)
    return m


_PROGRAM_CACHE = {}


def _get_program(with_qkv_bias, with_ln_bias, legalize=True):
    key = (with_qkv_bias, with_ln_bias, legalize)
    if key not in _PROGRAM_CACHE:
        nc = build_program(with_qkv_bias, with_ln_bias)
        if legalize:
            legalize_sync_waits(nc, 1)
        _PROGRAM_CACHE[key] = nc
    return _PROGRAM_CACHE[key]


def _run(inputs, trace=False):
    x = np.asarray(inputs["x"], np.float32)
    Wqkv_w = np.asarray(inputs["Wqkv_w"], np.float32)
    Wqkv_b = np.asarray(inputs["Wqkv_b"], np.float32)
    qn_g = np.asarray(inputs["qn_g"], np.float32)
    qn_b = np.asarray(inputs["qn_b"], np.float32)
    kn_g = np.asarray(inputs["kn_g"], np.float32)
    kn_b = np.asarray(inputs["kn_b"], np.float32)
    out_w = np.asarray(inputs["out_w"], np.float32)
    out_b = np.asarray(inputs["out_b"], np.float32)

    import time as _time

    _t = _time.time()
    in_maps = [
        _prep_core_inputs(c, x, Wqkv_w, Wqkv_b, qn_g, qn_b, kn_g, kn_b, out_w)
        for c in range(8)
    ]
    print(f"[kernel] host prep {_time.time()-_t:.1f}s", flush=True)
    _t = _time.time()
    nc = _get_program("brow" in in_maps[0], "tln" in in_maps[0])
    print(f"[kernel] program {_time.time()-_t:.1f}s", flush=True)
    _t = _time.time()
    res = run_bass_kernel_spmd(nc, in_maps, list(range(8)), trace=trace)
    print(f"[kernel] run {_time.time()-_t:.1f}s", flush=True)

    B = x.shape[0]
    bv = Wqkv_b[2 * DIM : 3 * DIM]
    out_bias = out_b + out_w @ bv
    out = np.empty((B, N, DIM), np.float32)
    for b in range(B):
        out[b] = res.results[2 * b]["outp"] + res.results[2 * b + 1]["outp"] + out_bias
    return out, res


def kernel(**inputs):
    out, _ = _run(inputs, trace=False)
    return out


# revision 25
# speedup vs baseline: 1.4724x; 1.4724x over previous
"""Trainium2 Bass kernel for nn_Attention_11836929868370.

8-core sharding: core c -> batch b = c//2, head group hg = c%2 (4 of 8 heads).
Each core computes its 4 heads' attention and a partial output projection;
the host sums the two partials per batch and adds the output bias.

v2 pipeline (all matmuls bf16, accumulation fp32 in PSUM):
  B1. qkv = xT.T @ WqkvT with host-built "mean columns" so the per-head LN
      mean comes out of the matmul for free; v evacuated on the SCALAR
      engine (ACT idles in phase B), LN stats on DVE.
  rs. batched Newton rsqrt on DVE (quake seed + 3 iterations).
  B2. RoPE via the rotate-half trick; BOTH LN scales (rs_q with the 1/sqrt(d)
      factor from the tables, rs_k plain) are folded into qT/kT so the exp
      stage needs no per-partition scale and can be split across engines.
      qT transposed on the sync DGE queue, kT on the gpsimd DGE queue.
  C.  per (q-block, head-pair): S^T = k'' @ q''.T for BOTH heads of the pair
      CONCURRENTLY as 64-row tile_position row-tiles (head0 rows 0:63,
      head1 rows 64:127 -> different PSUM banks; PE overlaps them, 2x score
      throughput). exp: most tiles on ACT (table exp), every third tile as a
      1-op Schraudolph int16-bitcast-bf16 exp on DVE (max 3.3% elementwise,
      diluted ~3x in softmax mixing; validated vs reference). PV with
      stationary [v_h | ones] (65 rows; row 64 = softmax denominator).
      Normalize: den row chunk-transposed on PE -> [128, 16] column layout,
      ONE cheap DVE reciprocal (vs 8-cycle/elem iterative on a 1024-wide
      row), transposed back, bf16 rec row bounced through a DRAM scratch and
      re-read with a stride-0 partition broadcast -> [64, 1024]; one DVE
      multiply per (head, half) normalizes straight out of PSUM.
  D.  out = oT.T @ WoT partial projection per 128-row tile; PSUM evacuated
      alternately on ACT/DVE, stored from the sync queue. Host adds out_b
      (+ the v-bias contribution, which commutes through softmax averaging).
"""

import sys

if "/opt/trn_rl_repo" not in sys.path:
    sys.path.insert(0, "/opt/trn_rl_repo")

from contextlib import ExitStack

import math
import ml_dtypes
import numpy as np

import concourse.bass as bass
import concourse.mybir as mybir
import concourse.tile as tile
from concourse.bass_utils import run_bass_kernel_spmd

BF16 = mybir.dt.bfloat16
F32 = mybir.dt.float32
I32 = mybir.dt.int32
I16 = mybir.dt.int16

DIM, NH, HD = 512, 8, 64
N = 2048
EPS = 1e-6
THETA = 10000.0
NT = N // 128          # 16 n-tiles
CT = DIM // 128        # 4 c-tiles
NHC = 4                # heads per core
QB = 4                 # q blocks of 512
KT = NT                # key tiles
QBW = N // QB          # 512
RSQRT_MAGIC = float(0x5F3759DF)
# Schraudolph exp in bf16-bit domain: e ~= bitcast_bf16(int16(A16*x + B16))
A16 = 2.0 ** 7 / math.log(2.0)
B16 = 127.0 * 2 ** 7 - 5.6
EXP_DVE_MOD = 3        # every EXP_DVE_MOD-th exp tile goes to DVE


# ---------------------------------------------------------------------------
# sync-wait legalization: this walrus build rejects >1 sync wait per
# instruction ("Too many sync wait commands"), while Tile's sem assignment
# emits several. Excess waits are hoisted onto NoOps placed immediately
# before the instruction on the same engine, which preserves ordering.
# ---------------------------------------------------------------------------

def legalize_sync_waits(nc, max_waits=1):
    n = 0
    for fn in nc.m.functions:
        for bb in fn.blocks:
            new_insts = []
            for inst in bb.instructions:
                si = inst.sync_info
                if si is not None and si.on_wait and len(si.on_wait) > max_waits:
                    movable = [w for w in si.on_wait if w.wait_reg is None]
                    pinned = [w for w in si.on_wait if w.wait_reg is not None]
                    budget = max(max_waits - len(pinned), 0)
                    cut = len(movable) - budget
                    keep, excess = movable[cut:], movable[:cut]
                    for i in range(0, len(excess), max_waits):
                        nop = mybir.InstNoOp(
                            name=f"I-waitsplit-{n}",
                            engine=inst.engine,
                            text_hint="waitsplit",
                            sync_info=mybir.SyncInfo(
                                on_wait=excess[i : i + max_waits], on_update=[]
                            ),
                        )
                        n += 1
                        new_insts.append(nop)
                    si.on_wait = keep + pinned
                new_insts.append(inst)
            bb.instructions[:] = new_insts
    return n


# ---------------------------------------------------------------------------
# device program
# ---------------------------------------------------------------------------

def build_program(with_qkv_bias=False, with_ln_bias=False):
    nc = bass.Bass("TRN2", target_bir_lowering=False, debug=False, num_devices=8)

    # [128, CT, 2048]: x transposed (c on partitions) and cast to bf16
    xT_d = nc.dram_tensor("xT", [128, CT * N], BF16, kind="ExternalInput").ap()
    # [128, CT, 776]: wq(256 perm) | wk(256 perm) | wv(256) | mu_q(4) | mu_k(4)
    wq_d = nc.dram_tensor("wqkvT", [128, CT * 776], BF16, kind="ExternalInput").ap()
    wo_d = nc.dram_tensor("woT", [64, NHC * DIM], BF16, kind="ExternalInput").ap()
    # [128, NT, 256]: C2q | S2q | C2k | S2k  (gains, q-scale folded in)
    tab_d = nc.dram_tensor("tab", [128, NT * 256], BF16, kind="ExternalInput").ap()
    id_d = nc.dram_tensor("ident", [128, 128], F32, kind="ExternalInput").ap()
    sel_d = nc.dram_tensor("sel", [8, 8 * 64], BF16, kind="ExternalInput").ap()
    if with_qkv_bias:
        b_d = nc.dram_tensor("brow", [1, 776], BF16, kind="ExternalInput").ap()
    if with_ln_bias:
        tln_d = nc.dram_tensor("tln", [128, NT * 512], BF16, kind="ExternalInput").ap()
    out_d = nc.dram_tensor("outp", [N, DIM], F32, kind="ExternalOutput").ap()

    with tile.TileContext(nc) as tc, ExitStack() as ctx:
        consts = ctx.enter_context(tc.tile_pool(name="consts", bufs=1))
        pers = ctx.enter_context(tc.tile_pool(name="pers", bufs=1))
        stage = ctx.enter_context(tc.tile_pool(name="stage", bufs=4))
        small = ctx.enter_context(tc.tile_pool(name="small", bufs=4))
        exps = ctx.enter_context(tc.tile_pool(name="exps", bufs=10))
        ps = ctx.enter_context(tc.tile_pool(name="ps", bufs=1, space="PSUM"))

        def ps_big(name):
            # [128, 1024] f32 = 2 PSUM banks; shared ring (scores/qkv/den/proj)
            return ps.tile([128, 1024], F32, tag="s", name=name, bufs=2)

        # constants
        xT_sb = consts.tile([128, CT, N], BF16)
        nc.sync.dma_start(xT_sb[:], xT_d.rearrange("p (t f) -> p t f", t=CT))
        wq_sb = consts.tile([128, CT, 776], BF16)
        nc.sync.dma_start(wq_sb[:], wq_d.rearrange("p (t f) -> p t f", t=CT))
        wo_sb = consts.tile([64, NHC, DIM], BF16)
        nc.sync.dma_start(wo_sb[:], wo_d.rearrange("p (t f) -> p t f", t=NHC))
        tab_sb = consts.tile([128, NT, 256], BF16)
        nc.sync.dma_start(tab_sb[:], tab_d.rearrange("p (t f) -> p t f", t=NT))
        ident = consts.tile([128, 128], F32)
        nc.sync.dma_start(ident[:], id_d)
        identb = consts.tile([128, 128], BF16)
        nc.vector.tensor_copy(identb[:], ident[:])
        # selector rows for the reciprocal-replicate matmuls: sel[p, c, j] = (p==c)
        sel_sb = consts.tile([8, 8, 64], BF16)
        nc.sync.dma_start(sel_sb[:], sel_d.rearrange("p (c j) -> p c j", c=8))
        if with_qkv_bias:
            b_sb = consts.tile([1, 776], BF16)
            nc.sync.dma_start(b_sb[:], b_d)
            ones_sb = consts.tile([1, 128], BF16)
            nc.vector.memset(ones_sb[:], 1.0)
        if with_ln_bias:
            tln_sb = consts.tile([128, NT, 512], BF16)
            nc.sync.dma_start(tln_sb[:], tln_d.rearrange("p (t f) -> p t f", t=NT))

        # persistent intermediates
        qT = [pers.tile([128, N], BF16, name=f"qT{i}") for i in range(2)]
        kT = [pers.tile([128, N], BF16, name=f"kT{i}") for i in range(2)]
        oT = [pers.tile([64, N], BF16, name=f"oTh{i}") for i in range(NHC)]
        # v with a ones column per head: PV row 64 is the softmax denominator
        v_sb = pers.tile([128, KT, NHC, 65], BF16)
        t_all = pers.tile([128, NT, 8, HD], BF16)
        ssq_all = pers.tile([128, NT, 8], F32)
        rs_sb = pers.tile([128, NT, 8], F32)
        rs_bf = pers.tile([128, NT, 8], BF16)

        nc.vector.memset(v_sb[:, :, :, 64], 1.0)

        # ---- phase B1: qkv matmuls + stats ----
        for nt in range(NT):
            qkv_ps = ps_big("qkv")
            for j0, j1 in ((0, 512), (512, 776)):
                for ct in range(CT):
                    nc.tensor.matmul(
                        qkv_ps[:, j0:j1],
                        lhsT=xT_sb[:, ct, nt * 128 : (nt + 1) * 128],
                        rhs=wq_sb[:, ct, j0:j1],
                        start=(ct == 0),
                        stop=(ct == CT - 1) and not with_qkv_bias,
                    )
                if with_qkv_bias:
                    nc.tensor.matmul(
                        qkv_ps[:, j0:j1],
                        lhsT=ones_sb[:],
                        rhs=b_sb[:, j0:j1],
                        start=False,
                        stop=True,
                    )
            mu = small.tile([128, 8], F32, name="mu")
            nc.scalar.copy(mu[:], qkv_ps[:, 768:776])
            nc.vector.tensor_tensor(
                t_all[:, nt],
                qkv_ps[:, 0:512].rearrange("p (h d) -> p h d", h=8),
                mu.unsqueeze(2).to_broadcast((128, 8, HD)),
                mybir.AluOpType.subtract,
            )
            # v evacuation on the scalar engine (ACT is idle during phase B)
            nc.scalar.copy(
                v_sb[:, nt, :, 0:64],
                qkv_ps[:, 512:768].rearrange("p (h d) -> p h d", h=NHC),
            )
            sq = stage.tile([128, 8, HD], BF16, name="sq")
            nc.vector.tensor_mul(sq[:], t_all[:, nt], t_all[:, nt])
            nc.vector.tensor_reduce(
                ssq_all[:, nt], sq[:], axis=mybir.AxisListType.X, op=mybir.AluOpType.add
            )

        # ---- batched rsqrt on DVE: rs = 1/sqrt(ssq/HD + eps) ----
        FLAT = NT * 8
        d_t = pers.tile([128, FLAT], F32, name="rsq_d")
        nc.vector.tensor_scalar(
            d_t[:], ssq_all.rearrange("p a b -> p (a b)"), 1.0 / HD, EPS,
            mybir.AluOpType.mult, mybir.AluOpType.add,
        )
        fi = small.tile([128, FLAT], F32, name="rsq_fi")
        nc.vector.tensor_copy(fi[:], d_t[:].bitcast(I32))  # int32 -> f32 convert
        nc.vector.tensor_scalar(
            fi[:], fi[:], -0.5, RSQRT_MAGIC, mybir.AluOpType.mult, mybir.AluOpType.add
        )
        yi = small.tile([128, FLAT], I32, name="rsq_yi")
        nc.vector.tensor_copy(yi[:], fi[:])  # f32 -> int32 convert
        y = yi[:].bitcast(F32)
        h_t = small.tile([128, FLAT], F32, name="rsq_h")
        for _ in range(3):
            nc.vector.tensor_mul(h_t[:], y, y)
            nc.vector.tensor_mul(h_t[:], h_t[:], d_t[:])
            nc.vector.tensor_scalar(
                h_t[:], h_t[:], -0.5, 1.5, mybir.AluOpType.mult, mybir.AluOpType.add
            )
            nc.vector.tensor_mul(y, y, h_t[:])
        nc.vector.tensor_copy(rs_sb.rearrange("p a b -> p (a b)"), y)
        nc.vector.tensor_copy(
            rs_bf.rearrange("p a b -> p (a b)"), rs_sb.rearrange("p a b -> p (a b)")
        )

        # ---- phase B2: rope + LN scales + transposes ----
        for nt in range(NT):
            t3 = t_all[:, nt]  # [p, 8, 64] bf16
            u = stage.tile([128, 2, 4, HD], BF16, name="u")
            w = stage.tile([128, 2, 4, HD], BF16, name="w")
            t4 = t3.rearrange("p (s h) d -> p s h d", s=2)
            # tab per nt: [C2q(64) | S2q(64) | C2k(64) | S2k(64)]; the q/k
            # sides sit 128 apart, so one strided-broadcast AP covers both
            tabs = tab_sb[:, nt].rearrange("p (s f) -> p s f", s=2)
            nc.vector.tensor_mul(
                u[:],
                t4,
                tabs[:, :, 0:64].unsqueeze(2).to_broadcast((128, 2, 4, HD)),
            )
            for half in (0, 1):
                d_out = slice(half * 32, half * 32 + 32)
                d_in = slice((1 - half) * 32, (1 - half) * 32 + 32)
                nc.vector.tensor_mul(
                    w[:, :, :, d_out],
                    t4[:, :, :, d_in],
                    tabs[:, :, 64 + half * 32 : 96 + half * 32]
                    .unsqueeze(2)
                    .to_broadcast((128, 2, 4, 32)),
                )
            u = u.rearrange("p s h d -> p (s h) d")
            w = w.rearrange("p s h d -> p (s h) d")
            qk2 = stage.tile([128, 8, HD], BF16, name="qk2", tag="qk2", bufs=3)
            nc.vector.tensor_add(qk2[:], u[:], w[:])
            if with_ln_bias:
                nc.vector.tensor_add(
                    qk2[:], qk2[:],
                    tln_sb[:, nt, :].rearrange("p (h d) -> p h d", h=8),
                )
            # both LN scales folded here (q: rs_q, with 1/sqrt(d) in tables;
            # k: rs_k) so exp needs no scale operand
            nc.vector.tensor_mul(
                qk2[:],
                qk2[:],
                rs_bf[:, nt, :].unsqueeze(2).to_broadcast((128, 8, HD)),
            )
            # transposes on the PE (is_transpose matmuls into PSUM, evacuated
            # on the scalar engine): keeps the DMA queues out of the loop and
            # the dependencies on engine semaphores
            flat = qk2.rearrange("p h d -> p (h d)")
            tps = ps_big("tp").bitcast(BF16)  # [128, 2048] bf16 view
            for c, (dstT, col) in enumerate(
                ((qT[0], 0), (qT[1], 128), (kT[0], 256), (kT[1], 384))
            ):
                nc.tensor.transpose(
                    tps[:, c * 128 : (c + 1) * 128],
                    flat[:, col : col + 128],
                    identb[:],
                )
                nc.scalar.copy(
                    dstT[:, nt * 128 : (nt + 1) * 128], tps[:, c * 128 : (c + 1) * 128]
                )

        # ---- phase C: attention ----
        # per (512-wide q-block, head-pair): one scores tile per kt holds BOTH
        # heads ([h0 512q | h1 512q] — different PSUM banks, so the two
        # row-tiled 64-contraction matmuls run concurrently); one FD-1024 exp
        # instruction per kt covers both heads. Because engine streams are
        # strictly in-order, PV is emitted TWO kt behind scores (so PV's wait
        # on exp never blocks the next scores issue), and each unit's
        # normalize + the q-block's projection are deferred and drip-fed into
        # the NEXT unit's kt loop, one chunk per kt, to keep the PE stream
        # dense (HAM stays at full clock only without idle gaps).
        pend = []

        def emit_norm(pair, qb, den_sb, oraw):
            # den_sb/oraw already evacuated to SBUF at unit end; this chain
            # runs as deferred chunks inside the NEXT unit's kt loop.
            unit = qb * 2 + pair
            qsl = slice(qb * QBW, (qb + 1) * QBW)

            dtile = ps.tile([128, QBW], F32, tag="dt", name=f"dt{unit}", bufs=1)
            rec_col = small.tile([128, 8], F32, name="rcol", tag="rcol", bufs=2)
            rec_row = small.tile([8, 128], BF16, name="rrow", tag="rrow", bufs=2)
            rec_rep = stage.tile([64, 2, QBW], BF16, name="rrep", tag="rrep", bufs=2)

            def c1():
                for c in range(8):
                    nc.tensor.transpose(
                        dtile[:, c : c + 1],
                        den_sb[:, c * 128 : (c + 1) * 128],
                        ident[0:1, 0:1],
                    )

            def c2():
                nc.vector.reciprocal(rec_col[:], dtile[:, 0:8])

            def c3():
                nc.tensor.transpose(dtile[0:8, 128:256], rec_col[:], ident[:])

            def c4():
                nc.vector.tensor_copy(rec_row[:], dtile[0:8, 128:256])

            def c5():
                # replicate each 128-chunk of rec across 64 partitions via
                # selector matmuls into the same 1-bank dt tile (h0 then h1)
                for c in range(4):
                    nc.tensor.matmul(
                        dtile[0:64, c * 128 : (c + 1) * 128],
                        lhsT=sel_sb[:, c, :],
                        rhs=rec_row[:],
                        start=True,
                        stop=True,
                    )
                nc.scalar.copy(rec_rep[:, 0, :], dtile[0:64, :])
                for c in range(4):
                    nc.tensor.matmul(
                        dtile[0:64, c * 128 : (c + 1) * 128],
                        lhsT=sel_sb[:, 4 + c, :],
                        rhs=rec_row[:],
                        start=True,
                        stop=True,
                    )
                nc.scalar.copy(rec_rep[:, 1, :], dtile[0:64, :])

            def c6():
                for hh in range(2):
                    h = pair * 2 + hh
                    nc.vector.tensor_mul(
                        oT[h][:, qsl], oraw[:, hh, :], rec_rep[:, hh, :]
                    )

            return [c1, c2, c3, c4, c5, c6]

        def emit_proj(qb):
            chunks = []
            for i, nt in enumerate(range(qb * (NT // QB), (qb + 1) * (NT // QB))):
                def cproj(nt=nt, i=i):
                    op = ps_big("op")
                    for h in range(NHC):
                        nc.tensor.matmul(
                            op[:, 0:512],
                            lhsT=oT[h][:, nt * 128 : (nt + 1) * 128],
                            rhs=wo_sb[:, h, :],
                            start=(h == 0),
                            stop=(h == NHC - 1),
                        )
                    ot = stage.tile([128, DIM], F32, name="ot", tag="ot", bufs=3)
                    if i % 2 == 0:
                        nc.scalar.copy(ot[:], op[:, 0:512])
                    else:
                        nc.vector.tensor_copy(ot[:], op[:, 0:512])
                    nc.sync.dma_start(out_d[nt * 128 : (nt + 1) * 128, :], ot[:])

                chunks.append(cproj)
            return chunks

        for qb in range(QB):
            for pair in range(2):
                unit = qb * 2 + pair
                qsl = slice(qb * QBW, (qb + 1) * QBW)
                # PV accumulators: hh -> [65 used, 512] in a 1-bank tile
                oTs = [
                    ps.tile([128, QBW], F32, tag="o", name=f"oT{unit}_{i}", bufs=3)
                    for i in range(2)
                ]
                ebs = {}
                den_sb = small.tile([1, 2 * QBW], F32, name="den", tag="den", bufs=2)
                oraw = stage.tile([64, 2, QBW], BF16, name="oraw", tag="oraw", bufs=2)

                def emit_sc(kt):
                    sS = ps_big(f"s_{unit}_{kt}")
                    for hh in range(2):
                        dsl = slice(hh * 64, hh * 64 + 64)
                        nc.tensor.matmul(
                            sS[:, hh * QBW : (hh + 1) * QBW],
                            lhsT=kT[pair][dsl, kt * 128 : (kt + 1) * 128],
                            rhs=qT[pair][dsl, qsl],
                            start=True,
                            stop=True,
                        )
                    e_sb = exps.tile([128, 2 * QBW], I16, tag="expS", name="expS")
                    if (unit * KT + kt) % EXP_DVE_MOD == EXP_DVE_MOD - 1:
                        # Schraudolph exp: int16 bits of the bf16 result
                        nc.vector.tensor_scalar(
                            e_sb[:], sS[:], A16, B16,
                            mybir.AluOpType.mult, mybir.AluOpType.add,
                        )
                    else:
                        nc.scalar.activation(
                            e_sb[:].bitcast(BF16), sS[:],
                            mybir.ActivationFunctionType.Exp,
                        )
                    ebs[kt] = e_sb[:].bitcast(BF16)

                def emit_pv(kt):
                    eb = ebs.pop(kt)
                    for hh in range(2):
                        nc.tensor.matmul(
                            oTs[hh][0:65, :],
                            lhsT=v_sb[:, kt, pair * 2 + hh, :],
                            rhs=eb[:, hh * QBW : (hh + 1) * QBW],
                            start=(kt == 0),
                            stop=(kt == KT - 1),
                        )

                for kt in range(KT):
                    emit_sc(kt)
                    if kt >= 1 and pend:
                        pend.pop(0)()
                    if kt >= 2:
                        emit_pv(kt - 2)
                emit_pv(KT - 2)
                emit_pv(KT - 1)

                # evacuate den rows (ACT) and unnormalized oT (DVE) right away
                # so the PV accumulator banks free before the next unit's PVs
                for hh in range(2):
                    nc.scalar.copy(
                        den_sb[:, hh * QBW : (hh + 1) * QBW], oTs[hh][64:65, :]
                    )
                    nc.vector.tensor_copy(oraw[:, hh, :], oTs[hh][0:64, :])
                pend.extend(emit_norm(pair, qb, den_sb, oraw))
            pend.extend(emit_proj(qb))

        for f in pend:
            f()

    return nc


# ---------------------------------------------------------------------------
# host-side input prep
# ---------------------------------------------------------------------------

def _prep_core_inputs(c, x, Wqkv_w, Wqkv_b, qn_g, qn_b, kn_g, kn_b, out_w):
    bf16 = ml_dtypes.bfloat16
    b, hg = c // 2, c % 2
    heads = np.arange(4 * hg, 4 * hg + 4)
    perm = np.concatenate([np.arange(0, HD, 2), np.arange(1, HD, 2)])

    Wq = Wqkv_w[0 * DIM : 1 * DIM].reshape(NH, HD, DIM)[heads][:, perm, :]
    Wk = Wqkv_w[1 * DIM : 2 * DIM].reshape(NH, HD, DIM)[heads][:, perm, :]
    Wv = Wqkv_w[2 * DIM : 3 * DIM].reshape(NH, HD, DIM)[heads]
    WT = np.concatenate(
        [
            Wq.reshape(256, DIM).T,
            Wk.reshape(256, DIM).T,
            Wv.reshape(256, DIM).T,
            (Wq.sum(axis=1) / HD).T,
            (Wk.sum(axis=1) / HD).T,
        ],
        axis=1,
    )  # [512, 776]
    wqkvT = np.ascontiguousarray(
        WT.reshape(CT, 128, 776).transpose(1, 0, 2).reshape(128, CT * 776)
    ).astype(bf16)

    # x transposed to [c, n] and tiled [128, CT, N]
    xTn = x[b].T  # [512, 2048]
    xT = np.ascontiguousarray(
        xTn.reshape(CT, 128, N).transpose(1, 0, 2).reshape(128, CT * N)
    ).astype(bf16)

    inv = 1.0 / (THETA ** (np.arange(0, HD, 2, dtype=np.float64) / HD))
    ang = np.arange(N, dtype=np.float64)[:, None] * inv[None, :]
    cos = np.cos(ang)
    sin = np.sin(ang)
    C2 = np.concatenate([cos, cos], axis=1)
    S2 = np.concatenate([-sin, sin], axis=1)
    SH = lambda v: np.concatenate([v[HD // 2 :], v[: HD // 2]])
    sc = HD ** -0.5
    g_q, g_k = qn_g[perm], kn_g[perm]
    C2q = C2 * g_q[None, :] * sc
    S2q = S2 * SH(g_q)[None, :] * sc
    C2k = C2 * g_k[None, :]
    S2k = S2 * SH(g_k)[None, :]
    tabN = np.concatenate([C2q, S2q, C2k, S2k], axis=1)  # [N, 256]
    tab = np.ascontiguousarray(
        tabN.reshape(NT, 128, 256).transpose(1, 0, 2).reshape(128, NT * 256)
    ).astype(bf16)

    # per-head Wo^T blocks [64, 512], stacked along free: [64, NHC*512]
    Wo = out_w.reshape(DIM, NH, HD)[:, heads, :]  # [512, 4, 64]
    woT = np.ascontiguousarray(
        Wo.transpose(1, 2, 0).reshape(NHC, HD, DIM).transpose(1, 0, 2).reshape(HD, NHC * DIM)
    ).astype(bf16)

    sel = np.zeros((8, 8, HD), dtype=bf16)
    for cc in range(8):
        sel[cc, cc, :] = 1.0
    m = {
        "xT": xT,
        "wqkvT": wqkvT,
        "woT": woT,
        "tab": tab,
        "ident": np.eye(128, dtype=np.float32),
        "sel": sel.reshape(8, 8 * HD),
    }

    if np.any(Wqkv_b != 0):
        bq = Wqkv_b[0 * DIM : 1 * DIM].reshape(NH, HD)[heads][:, perm]
        bk = Wqkv_b[1 * DIM : 2 * DIM].reshape(NH, HD)[heads][:, perm]
        bv = Wqkv_b[2 * DIM : 3 * DIM].reshape(NH, HD)[heads]
        brow = np.concatenate(
            [bq.ravel(), bk.ravel(), bv.ravel(), bq.mean(1), bk.mean(1)]
        )[None, :]
        m["brow"] = brow.astype(bf16)
    if np.any(qn_b != 0) or np.any(kn_b != 0):
        b_q, b_k = qn_b[perm], kn_b[perm]
        Tq = (C2 * b_q[None, :] + S2 * SH(b_q)[None, :]) * sc
        Tk = C2 * b_k[None, :] + S2 * SH(b_k)[None, :]
        tlnN = np.concatenate([np.tile(Tq, (1, 4)), np.tile(Tk, (1, 4))], axis=1)
        m["tln"] = np.ascontiguousarray(
            tlnN.reshape(NT, 128, 512).transpose(1, 0, 2).reshape(128, NT * 512)
        ).astype(bf16)
    return m


_PROGRAM_CACHE = {}


def _get_program(with_qkv_bias, with_ln_bias, legalize=True):
    key = (with_qkv_bias, with_ln_bias, legalize)
    if key not in _PROGRAM_CACHE:
        nc = build_program(with_qkv_bias, with_ln_bias)
        if legalize:
            legalize_sync_waits(nc, 1)
        _PROGRAM_CACHE[key] = nc
    return _PROGRAM_CACHE[key]


def _run(inputs, trace=False):
    x = np.asarray(inputs["x"], np.float32)
    Wqkv_w = np.asarray(inputs["Wqkv_w"], np.float32)
    Wqkv_b = np.asarray(inputs["Wqkv_b"], np.float32)
    qn_g = np.asarray(inputs["qn_g"], np.float32)
    qn_b = np.asarray(inputs["qn_b"], np.float32)
    kn_g = np.asarray(inputs["kn_g"], np.float32)
    kn_b = np.asarray(inputs["kn_b"], np.float32)
    out_w = np.asarray(inputs["out_w"], np.float32)
    out_b = np.asarray(inputs["out_b"], np.float32)

    import time as _time

    _t = _time.time()
    in_maps = [
        _prep_core_inputs(c, x, Wqkv_w, Wqkv_b, qn_g, qn_b, kn_g, kn_b, out_w)
        for c in range(8)
    ]
    print(f"[kernel] host prep {_time.time()-_t:.1f}s", flush=True)
    _t = _time.time()
    nc = _get_program("brow" in in_maps[0], "tln" in in_maps[0])
    print(f"[kernel] program {_time.time()-_t:.1f}s", flush=True)
    _t = _time.time()
    res = run_bass_kernel_spmd(nc, in_maps, list(range(8)), trace=trace)
    print(f"[kernel] run {_time.time()-_t:.1f}s", flush=True)

    B = x.shape[0]
    bv = Wqkv_b[2 * DIM : 3 * DIM]
    out_bias = out_b + out_w @ bv
    out = np.empty((B, N, DIM), np.float32)
    for b in range(B):
        out[b] = res.results[2 * b]["outp"] + res.results[2 * b + 1]["outp"] + out_bias
    return out, res


def kernel(**inputs):
    out, _ = _run(inputs, trace=False)
    return out


# revision 33
# speedup vs baseline: 1.5142x; 1.0284x over previous
"""Trainium2 Bass kernel for nn_Attention_11836929868370.

8-core sharding: core c -> batch b = c//2, head group hg = c%2 (4 of 8 heads).
Each core computes its 4 heads' attention and a partial output projection;
the host sums the two partials per batch and adds the output bias.

v2 pipeline (all matmuls bf16, accumulation fp32 in PSUM):
  B1. qkv = xT.T @ WqkvT with host-built "mean columns" so the per-head LN
      mean comes out of the matmul for free; v evacuated on the SCALAR
      engine (ACT idles in phase B), LN stats on DVE.
  rs. batched Newton rsqrt on DVE (quake seed + 3 iterations).
  B2. RoPE via the rotate-half trick; BOTH LN scales (rs_q with the 1/sqrt(d)
      factor from the tables, rs_k plain) are folded into qT/kT so the exp
      stage needs no per-partition scale and can be split across engines.
      qT transposed on the sync DGE queue, kT on the gpsimd DGE queue.
  C.  per (q-block, head-pair): S^T = k'' @ q''.T for BOTH heads of the pair
      CONCURRENTLY as 64-row tile_position row-tiles (head0 rows 0:63,
      head1 rows 64:127 -> different PSUM banks; PE overlaps them, 2x score
      throughput). exp: most tiles on ACT (table exp), every third tile as a
      1-op Schraudolph int16-bitcast-bf16 exp on DVE (max 3.3% elementwise,
      diluted ~3x in softmax mixing; validated vs reference). PV with
      stationary [v_h | ones] (65 rows; row 64 = softmax denominator).
      Normalize: den row chunk-transposed on PE -> [128, 16] column layout,
      ONE cheap DVE reciprocal (vs 8-cycle/elem iterative on a 1024-wide
      row), transposed back, bf16 rec row bounced through a DRAM scratch and
      re-read with a stride-0 partition broadcast -> [64, 1024]; one DVE
      multiply per (head, half) normalizes straight out of PSUM.
  D.  out = oT.T @ WoT partial projection per 128-row tile; PSUM evacuated
      alternately on ACT/DVE, stored from the sync queue. Host adds out_b
      (+ the v-bias contribution, which commutes through softmax averaging).
"""

import sys

if "/opt/trn_rl_repo" not in sys.path:
    sys.path.insert(0, "/opt/trn_rl_repo")

from contextlib import ExitStack

import math
import ml_dtypes
import numpy as np

import concourse.bass as bass
import concourse.mybir as mybir
import concourse.tile as tile
from concourse.bass_utils import run_bass_kernel_spmd

BF16 = mybir.dt.bfloat16
F32 = mybir.dt.float32
I32 = mybir.dt.int32
I16 = mybir.dt.int16

DIM, NH, HD = 512, 8, 64
N = 2048
EPS = 1e-6
THETA = 10000.0
NT = N // 128          # 16 n-tiles
CT = DIM // 128        # 4 c-tiles
NHC = 4                # heads per core
QB = 4                 # q blocks of 512
KT = NT                # key tiles
QBW = N // QB          # 512
RSQRT_MAGIC = float(0x5F3759DF)
# Schraudolph exp in bf16-bit domain: e ~= bitcast_bf16(int16(A16*x + B16))
A16 = 2.0 ** 7 / math.log(2.0)
B16 = 127.0 * 2 ** 7 - 5.6
EXP_DVE_MOD = 2        # every EXP_DVE_MOD-th exp tile goes to DVE


# ---------------------------------------------------------------------------
# sync-wait legalization: this walrus build rejects >1 sync wait per
# instruction ("Too many sync wait commands"), while Tile's sem assignment
# emits several. Excess waits are hoisted onto NoOps placed immediately
# before the instruction on the same engine, which preserves ordering.
# ---------------------------------------------------------------------------

def legalize_sync_waits(nc, max_waits=1):
    n = 0
    for fn in nc.m.functions:
        for bb in fn.blocks:
            new_insts = []
            for inst in bb.instructions:
                si = inst.sync_info
                if si is not None and si.on_wait and len(si.on_wait) > max_waits:
                    movable = [w for w in si.on_wait if w.wait_reg is None]
                    pinned = [w for w in si.on_wait if w.wait_reg is not None]
                    budget = max(max_waits - len(pinned), 0)
                    cut = len(movable) - budget
                    keep, excess = movable[cut:], movable[:cut]
                    for i in range(0, len(excess), max_waits):
                        nop = mybir.InstNoOp(
                            name=f"I-waitsplit-{n}",
                            engine=inst.engine,
                            text_hint="waitsplit",
                            sync_info=mybir.SyncInfo(
                                on_wait=excess[i : i + max_waits], on_update=[]
                            ),
                        )
                        n += 1
                        new_insts.append(nop)
                    si.on_wait = keep + pinned
                new_insts.append(inst)
            bb.instructions[:] = new_insts
    return n


# ---------------------------------------------------------------------------
# device program
# ---------------------------------------------------------------------------

def build_program(with_qkv_bias=False, with_ln_bias=False):
    nc = bass.Bass("TRN2", target_bir_lowering=False, debug=False, num_devices=8)

    # [128, CT, 2048]: x transposed (c on partitions) and cast to bf16
    xT_d = nc.dram_tensor("xT", [128, CT * N], BF16, kind="ExternalInput").ap()
    # [128, CT, 768]: wq(256 perm) | wk(256 perm) | wv(256); q/k rows are
    # mean-centered per head on the host, so the matmul emits t = qk - mu
    wq_d = nc.dram_tensor("wqkvT", [128, CT * 768], BF16, kind="ExternalInput").ap()
    wo_d = nc.dram_tensor("woT", [64, NHC * DIM], BF16, kind="ExternalInput").ap()
    # [128, NT, 256]: C2q | S2q | C2k | S2k  (gains, q-scale folded in)
    tab_d = nc.dram_tensor("tab", [128, NT * 256], BF16, kind="ExternalInput").ap()
    id_d = nc.dram_tensor("ident", [128, 128], F32, kind="ExternalInput").ap()
    sel_d = nc.dram_tensor("sel", [8, 8 * 64], BF16, kind="ExternalInput").ap()
    if with_qkv_bias:
        b_d = nc.dram_tensor("brow", [1, 768], BF16, kind="ExternalInput").ap()
    if with_ln_bias:
        tln_d = nc.dram_tensor("tln", [128, NT * 512], BF16, kind="ExternalInput").ap()
    out_d = nc.dram_tensor("outp", [N, DIM], F32, kind="ExternalOutput").ap()

    with tile.TileContext(nc) as tc, ExitStack() as ctx:
        consts = ctx.enter_context(tc.tile_pool(name="consts", bufs=1))
        pers = ctx.enter_context(tc.tile_pool(name="pers", bufs=1))
        stage = ctx.enter_context(tc.tile_pool(name="stage", bufs=4))
        small = ctx.enter_context(tc.tile_pool(name="small", bufs=4))
        exps = ctx.enter_context(tc.tile_pool(name="exps", bufs=10))
        ps = ctx.enter_context(tc.tile_pool(name="ps", bufs=1, space="PSUM"))

        def ps_big(name):
            # [128, 1024] f32 = 2 PSUM banks; shared ring (scores/qkv/den/proj)
            return ps.tile([128, 1024], F32, tag="s", name=name, bufs=2)

        # constants
        xT_sb = consts.tile([128, CT, N], BF16)
        nc.sync.dma_start(xT_sb[:], xT_d.rearrange("p (t f) -> p t f", t=CT))
        wq_sb = consts.tile([128, CT, 768], BF16)
        nc.sync.dma_start(wq_sb[:], wq_d.rearrange("p (t f) -> p t f", t=CT))
        wo_sb = consts.tile([64, NHC, DIM], BF16)
        nc.sync.dma_start(wo_sb[:], wo_d.rearrange("p (t f) -> p t f", t=NHC))
        tab_sb = consts.tile([128, NT, 256], BF16)
        nc.sync.dma_start(tab_sb[:], tab_d.rearrange("p (t f) -> p t f", t=NT))
        ident = consts.tile([128, 128], F32)
        nc.sync.dma_start(ident[:], id_d)
        identb = consts.tile([128, 128], BF16)
        nc.vector.tensor_copy(identb[:], ident[:])
        # selector rows for the reciprocal-replicate matmuls: sel[p, c, j] = (p==c)
        sel_sb = consts.tile([8, 8, 64], BF16)
        nc.sync.dma_start(sel_sb[:], sel_d.rearrange("p (c j) -> p c j", c=8))
        if with_qkv_bias:
            b_sb = consts.tile([1, 768], BF16)
            nc.sync.dma_start(b_sb[:], b_d)
            ones_sb = consts.tile([1, 128], BF16)
            nc.vector.memset(ones_sb[:], 1.0)
        if with_ln_bias:
            tln_sb = consts.tile([128, NT, 512], BF16)
            nc.sync.dma_start(tln_sb[:], tln_d.rearrange("p (t f) -> p t f", t=NT))

        # persistent intermediates
        qT = [pers.tile([128, N], BF16, name=f"qT{i}") for i in range(2)]
        kT = [pers.tile([128, N], BF16, name=f"kT{i}") for i in range(2)]
        oT = [pers.tile([64, N], BF16, name=f"oTh{i}") for i in range(NHC)]
        # v with a ones column per head: PV row 64 is the softmax denominator
        v_sb = pers.tile([128, KT, NHC, 65], BF16)
        t_all = pers.tile([128, NT, 8, HD], BF16)
        ssq_all = pers.tile([128, NT, 8], F32)
        rs_sb = pers.tile([128, NT, 8], F32)
        rs_bf = pers.tile([128, NT, 8], BF16)

        nc.vector.memset(v_sb[:, :, :, 64], 1.0)

        # ---- phase B1: qkv matmuls + stats ----
        for nt in range(NT):
            qkv_ps = ps_big("qkv")
            for j0, j1 in ((0, 512), (512, 768)):
                for ct in range(CT):
                    nc.tensor.matmul(
                        qkv_ps[:, j0:j1],
                        lhsT=xT_sb[:, ct, nt * 128 : (nt + 1) * 128],
                        rhs=wq_sb[:, ct, j0:j1],
                        start=(ct == 0),
                        stop=(ct == CT - 1) and not with_qkv_bias,
                    )
                if with_qkv_bias:
                    nc.tensor.matmul(
                        qkv_ps[:, j0:j1],
                        lhsT=ones_sb[:],
                        rhs=b_sb[:, j0:j1],
                        start=False,
                        stop=True,
                    )
            # t and v evacuation on the scalar engine (ACT idles in phase B)
            nc.scalar.copy(t_all[:, nt].rearrange("p h d -> p (h d)"), qkv_ps[:, 0:512])
            nc.scalar.copy(
                v_sb[:, nt, :, 0:64],
                qkv_ps[:, 512:768].rearrange("p (h d) -> p h d", h=NHC),
            )
            sq = stage.tile([128, 8, HD], BF16, name="sq")
            nc.vector.tensor_mul(sq[:], t_all[:, nt], t_all[:, nt])
            nc.vector.tensor_reduce(
                ssq_all[:, nt], sq[:], axis=mybir.AxisListType.X, op=mybir.AluOpType.add
            )

        # ---- batched rsqrt on DVE: rs = 1/sqrt(ssq/HD + eps) ----
        FLAT = NT * 8
        d_t = pers.tile([128, FLAT], F32, name="rsq_d")
        nc.vector.tensor_scalar(
            d_t[:], ssq_all.rearrange("p a b -> p (a b)"), 1.0 / HD, EPS,
            mybir.AluOpType.mult, mybir.AluOpType.add,
        )
        fi = small.tile([128, FLAT], F32, name="rsq_fi")
        nc.vector.tensor_copy(fi[:], d_t[:].bitcast(I32))  # int32 -> f32 convert
        nc.vector.tensor_scalar(
            fi[:], fi[:], -0.5, RSQRT_MAGIC, mybir.AluOpType.mult, mybir.AluOpType.add
        )
        yi = small.tile([128, FLAT], I32, name="rsq_yi")
        nc.vector.tensor_copy(yi[:], fi[:])  # f32 -> int32 convert
        y = yi[:].bitcast(F32)
        h_t = small.tile([128, FLAT], F32, name="rsq_h")
        for _ in range(3):
            nc.vector.tensor_mul(h_t[:], y, y)
            nc.vector.tensor_mul(h_t[:], h_t[:], d_t[:])
            nc.vector.tensor_scalar(
                h_t[:], h_t[:], -0.5, 1.5, mybir.AluOpType.mult, mybir.AluOpType.add
            )
            nc.vector.tensor_mul(y, y, h_t[:])
        nc.vector.tensor_copy(rs_sb.rearrange("p a b -> p (a b)"), y)
        nc.vector.tensor_copy(
            rs_bf.rearrange("p a b -> p (a b)"), rs_sb.rearrange("p a b -> p (a b)")
        )

        # ---- phase B2: rope + LN scales + transposes ----
        for nt in range(NT):
            t3 = t_all[:, nt]  # [p, 8, 64] bf16
            u = stage.tile([128, 2, 4, HD], BF16, name="u")
            w = stage.tile([128, 2, 4, HD], BF16, name="w")
            t4 = t3.rearrange("p (s h) d -> p s h d", s=2)
            # tab per nt: [C2q(64) | S2q(64) | C2k(64) | S2k(64)]; the q/k
            # sides sit 128 apart, so one strided-broadcast AP covers both
            tabs = tab_sb[:, nt].rearrange("p (s f) -> p s f", s=2)
            nc.vector.tensor_mul(
                u[:],
                t4,
                tabs[:, :, 0:64].unsqueeze(2).to_broadcast((128, 2, 4, HD)),
            )
            for half in (0, 1):
                d_out = slice(half * 32, half * 32 + 32)
                d_in = slice((1 - half) * 32, (1 - half) * 32 + 32)
                nc.vector.tensor_mul(
                    w[:, :, :, d_out],
                    t4[:, :, :, d_in],
                    tabs[:, :, 64 + half * 32 : 96 + half * 32]
                    .unsqueeze(2)
                    .to_broadcast((128, 2, 4, 32)),
                )
            u = u.rearrange("p s h d -> p (s h) d")
            w = w.rearrange("p s h d -> p (s h) d")
            qk2 = stage.tile([128, 8, HD], BF16, name="qk2", tag="qk2", bufs=3)
            nc.vector.tensor_add(qk2[:], u[:], w[:])
            if with_ln_bias:
                nc.vector.tensor_add(
                    qk2[:], qk2[:],
                    tln_sb[:, nt, :].rearrange("p (h d) -> p h d", h=8),
                )
            # both LN scales folded here (q: rs_q, with 1/sqrt(d) in tables;
            # k: rs_k) so exp needs no scale operand
            nc.vector.tensor_mul(
                qk2[:],
                qk2[:],
                rs_bf[:, nt, :].unsqueeze(2).to_broadcast((128, 8, HD)),
            )
            # transposes on the PE (is_transpose matmuls into PSUM, evacuated
            # on the scalar engine): keeps the DMA queues out of the loop and
            # the dependencies on engine semaphores
            flat = qk2.rearrange("p h d -> p (h d)")
            tps = ps_big("tp").bitcast(BF16)  # [128, 2048] bf16 view
            for c, (dstT, col) in enumerate(
                ((qT[0], 0), (qT[1], 128), (kT[0], 256), (kT[1], 384))
            ):
                nc.tensor.transpose(
                    tps[:, c * 128 : (c + 1) * 128],
                    flat[:, col : col + 128],
                    identb[:],
                )
                nc.scalar.copy(
                    dstT[:, nt * 128 : (nt + 1) * 128], tps[:, c * 128 : (c + 1) * 128]
                )

        # ---- phase C: attention ----
        # per (512-wide q-block, head-pair): one scores tile per kt holds BOTH
        # heads ([h0 512q | h1 512q] — different PSUM banks, so the two
        # row-tiled 64-contraction matmuls run concurrently); one FD-1024 exp
        # instruction per kt covers both heads. Because engine streams are
        # strictly in-order, PV is emitted TWO kt behind scores (so PV's wait
        # on exp never blocks the next scores issue), and each unit's
        # normalize + the q-block's projection are deferred and drip-fed into
        # the NEXT unit's kt loop, one chunk per kt, to keep the PE stream
        # dense (HAM stays at full clock only without idle gaps).
        pend = []

        def emit_norm(pair, qb, den_sb, oraw):
            # den_sb/oraw already evacuated to SBUF at unit end; this chain
            # runs as deferred chunks inside the NEXT unit's kt loop.
            unit = qb * 2 + pair
            qsl = slice(qb * QBW, (qb + 1) * QBW)

            dtile = ps.tile([128, QBW], F32, tag="dt", name=f"dt{unit}", bufs=1)
            rec_col = small.tile([128, 8], F32, name="rcol", tag="rcol", bufs=2)
            rec_row = small.tile([8, 128], BF16, name="rrow", tag="rrow", bufs=2)
            rec_rep = stage.tile([64, 2, QBW], BF16, name="rrep", tag="rrep", bufs=2)

            def c1():
                for c in range(8):
                    nc.tensor.transpose(
                        dtile[:, c : c + 1],
                        den_sb[:, c * 128 : (c + 1) * 128],
                        ident[0:1, 0:1],
                    )

            def c2():
                nc.vector.reciprocal(rec_col[:], dtile[:, 0:8])

            def c3():
                nc.tensor.transpose(dtile[0:8, 128:256], rec_col[:], ident[:])

            def c4():
                nc.vector.tensor_copy(rec_row[:], dtile[0:8, 128:256])

            def c5():
                # replicate each 128-chunk of rec across 64 partitions via
                # selector matmuls into the same 1-bank dt tile (h0 then h1)
                for c in range(4):
                    nc.tensor.matmul(
                        dtile[0:64, c * 128 : (c + 1) * 128],
                        lhsT=sel_sb[:, c, :],
                        rhs=rec_row[:],
                        start=True,
                        stop=True,
                    )
                nc.scalar.copy(rec_rep[:, 0, :], dtile[0:64, :])
                for c in range(4):
                    nc.tensor.matmul(
                        dtile[0:64, c * 128 : (c + 1) * 128],
                        lhsT=sel_sb[:, 4 + c, :],
                        rhs=rec_row[:],
                        start=True,
                        stop=True,
                    )
                nc.scalar.copy(rec_rep[:, 1, :], dtile[0:64, :])

            def c6():
                for hh in range(2):
                    h = pair * 2 + hh
                    nc.vector.tensor_mul(
                        oT[h][:, qsl], oraw[:, hh, :], rec_rep[:, hh, :]
                    )

            return [c1, c2, c3, c4, c5, c6]

        def emit_proj(qb):
            chunks = []
            for i, nt in enumerate(range(qb * (NT // QB), (qb + 1) * (NT // QB))):
                def cproj(nt=nt, i=i):
                    op = ps_big("op")
                    for h in range(NHC):
                        nc.tensor.matmul(
                            op[:, 0:512],
                            lhsT=oT[h][:, nt * 128 : (nt + 1) * 128],
                            rhs=wo_sb[:, h, :],
                            start=(h == 0),
                            stop=(h == NHC - 1),
                        )
                    ot = stage.tile([128, DIM], F32, name="ot", tag="ot", bufs=3)
                    if i % 2 == 0:
                        nc.scalar.copy(ot[:], op[:, 0:512])
                    else:
                        nc.vector.tensor_copy(ot[:], op[:, 0:512])
                    nc.sync.dma_start(out_d[nt * 128 : (nt + 1) * 128, :], ot[:])

                chunks.append(cproj)
            return chunks

        for qb in range(QB):
            for pair in range(2):
                unit = qb * 2 + pair
                qsl = slice(qb * QBW, (qb + 1) * QBW)
                # PV accumulators: hh -> [65 used, 512] in a 1-bank tile
                oTs = [
                    ps.tile([128, QBW], F32, tag="o", name=f"oT{unit}_{i}", bufs=3)
                    for i in range(2)
                ]
                ebs = {}
                den_sb = small.tile([1, 2 * QBW], F32, name="den", tag="den", bufs=2)
                oraw = stage.tile([64, 2, QBW], BF16, name="oraw", tag="oraw", bufs=2)

                def emit_sc(kt):
                    sS = ps_big(f"s_{unit}_{kt}")
                    for hh in range(2):
                        dsl = slice(hh * 64, hh * 64 + 64)
                        nc.tensor.matmul(
                            sS[:, hh * QBW : (hh + 1) * QBW],
                            lhsT=kT[pair][dsl, kt * 128 : (kt + 1) * 128],
                            rhs=qT[pair][dsl, qsl],
                            start=True,
                            stop=True,
                        )
                    e_sb = exps.tile([128, 2 * QBW], I16, tag="expS", name="expS")
                    if (unit * KT + kt) % EXP_DVE_MOD == EXP_DVE_MOD - 1:
                        # Schraudolph exp: int16 bits of the bf16 result
                        nc.vector.tensor_scalar(
                            e_sb[:], sS[:], A16, B16,
                            mybir.AluOpType.mult, mybir.AluOpType.add,
                        )
                    else:
                        nc.scalar.activation(
                            e_sb[:].bitcast(BF16), sS[:],
                            mybir.ActivationFunctionType.Exp,
                        )
                    ebs[kt] = e_sb[:].bitcast(BF16)

                def emit_pv(kt):
                    eb = ebs.pop(kt)
                    for hh in range(2):
                        nc.tensor.matmul(
                            oTs[hh][0:65, :],
                            lhsT=v_sb[:, kt, pair * 2 + hh, :],
                            rhs=eb[:, hh * QBW : (hh + 1) * QBW],
                            start=(kt == 0),
                            stop=(kt == KT - 1),
                        )

                for kt in range(KT):
                    emit_sc(kt)
                    if kt >= 1 and pend:
                        pend.pop(0)()
                    if kt >= 2:
                        emit_pv(kt - 2)
                emit_pv(KT - 2)
                emit_pv(KT - 1)

                # evacuate den rows (ACT) and unnormalized oT (DVE) right away
                # so the PV accumulator banks free before the next unit's PVs
                for hh in range(2):
                    nc.scalar.copy(
                        den_sb[:, hh * QBW : (hh + 1) * QBW], oTs[hh][64:65, :]
                    )
                    nc.vector.tensor_copy(oraw[:, hh, :], oTs[hh][0:64, :])
                pend.extend(emit_norm(pair, qb, den_sb, oraw))
            pend.extend(emit_proj(qb))

        for f in pend:
            f()

    return nc


# ---------------------------------------------------------------------------
# host-side input prep
# ---------------------------------------------------------------------------

def _prep_core_inputs(c, x, Wqkv_w, Wqkv_b, qn_g, qn_b, kn_g, kn_b, out_w):
    bf16 = ml_dtypes.bfloat16
    b, hg = c // 2, c % 2
    heads = np.arange(4 * hg, 4 * hg + 4)
    perm = np.concatenate([np.arange(0, HD, 2), np.arange(1, HD, 2)])

    Wq = Wqkv_w[0 * DIM : 1 * DIM].reshape(NH, HD, DIM)[heads]
    Wk = Wqkv_w[1 * DIM : 2 * DIM].reshape(NH, HD, DIM)[heads]
    Wv = Wqkv_w[2 * DIM : 3 * DIM].reshape(NH, HD, DIM)[heads]
    # center q/k rows per head: the qkv matmul then directly emits t = qk - mu
    Wq = (Wq - Wq.mean(axis=1, keepdims=True))[:, perm, :]
    Wk = (Wk - Wk.mean(axis=1, keepdims=True))[:, perm, :]
    WT = np.concatenate(
        [
            Wq.reshape(256, DIM).T,
            Wk.reshape(256, DIM).T,
            Wv.reshape(256, DIM).T,
        ],
        axis=1,
    )  # [512, 768]
    wqkvT = np.ascontiguousarray(
        WT.reshape(CT, 128, 768).transpose(1, 0, 2).reshape(128, CT * 768)
    ).astype(bf16)

    # x transposed to [c, n] and tiled [128, CT, N]
    xTn = x[b].T  # [512, 2048]
    xT = np.ascontiguousarray(
        xTn.reshape(CT, 128, N).transpose(1, 0, 2).reshape(128, CT * N)
    ).astype(bf16)

    inv = 1.0 / (THETA ** (np.arange(0, HD, 2, dtype=np.float64) / HD))
    ang = np.arange(N, dtype=np.float64)[:, None] * inv[None, :]
    cos = np.cos(ang)
    sin = np.sin(ang)
    C2 = np.concatenate([cos, cos], axis=1)
    S2 = np.concatenate([-sin, sin], axis=1)
    SH = lambda v: np.concatenate([v[HD // 2 :], v[: HD // 2]])
    sc = HD ** -0.5
    g_q, g_k = qn_g[perm], kn_g[perm]
    C2q = C2 * g_q[None, :] * sc
    S2q = S2 * SH(g_q)[None, :] * sc
    C2k = C2 * g_k[None, :]
    S2k = S2 * SH(g_k)[None, :]
    tabN = np.concatenate([C2q, S2q, C2k, S2k], axis=1)  # [N, 256]
    tab = np.ascontiguousarray(
        tabN.reshape(NT, 128, 256).transpose(1, 0, 2).reshape(128, NT * 256)
    ).astype(bf16)

    # per-head Wo^T blocks [64, 512], stacked along free: [64, NHC*512]
    Wo = out_w.reshape(DIM, NH, HD)[:, heads, :]  # [512, 4, 64]
    woT = np.ascontiguousarray(
        Wo.transpose(1, 2, 0).reshape(NHC, HD, DIM).transpose(1, 0, 2).reshape(HD, NHC * DIM)
    ).astype(bf16)

    sel = np.zeros((8, 8, HD), dtype=bf16)
    for cc in range(8):
        sel[cc, cc, :] = 1.0
    m = {
        "xT": xT,
        "wqkvT": wqkvT,
        "woT": woT,
        "tab": tab,
        "ident": np.eye(128, dtype=np.float32),
        "sel": sel.reshape(8, 8 * HD),
    }

    if np.any(Wqkv_b != 0):
        bq = Wqkv_b[0 * DIM : 1 * DIM].reshape(NH, HD)[heads]
        bk = Wqkv_b[1 * DIM : 2 * DIM].reshape(NH, HD)[heads]
        bq = (bq - bq.mean(axis=1, keepdims=True))[:, perm]
        bk = (bk - bk.mean(axis=1, keepdims=True))[:, perm]
        bv = Wqkv_b[2 * DIM : 3 * DIM].reshape(NH, HD)[heads]
        brow = np.concatenate([bq.ravel(), bk.ravel(), bv.ravel()])[None, :]
        m["brow"] = brow.astype(bf16)
    if np.any(qn_b != 0) or np.any(kn_b != 0):
        b_q, b_k = qn_b[perm], kn_b[perm]
        Tq = (C2 * b_q[None, :] + S2 * SH(b_q)[None, :]) * sc
        Tk = C2 * b_k[None, :] + S2 * SH(b_k)[None, :]
        tlnN = np.concatenate([np.tile(Tq, (1, 4)), np.tile(Tk, (1, 4))], axis=1)
        m["tln"] = np.ascontiguousarray(
            tlnN.reshape(NT, 128, 512).transpose(1, 0, 2).reshape(128, NT * 512)
        ).astype(bf16)
    return m


_PROGRAM_CACHE = {}


def _get_program(with_qkv_bias, with_ln_bias, legalize=True):
    key = (with_qkv_bias, with_ln_bias, legalize)
    if key not in _PROGRAM_CACHE:
        nc = build_program(with_qkv_bias, with_ln_bias)
        if legalize:
            legalize_sync_waits(nc, 1)
        _PROGRAM_CACHE[key] = nc
    return _PROGRAM_CACHE[key]


def _run(inputs, trace=False):
    x = np.asarray(inputs["x"], np.float32)
    Wqkv_w = np.asarray(inputs["Wqkv_w"], np.float32)
    Wqkv_b = np.asarray(inputs["Wqkv_b"], np.float32)
    qn_g = np.asarray(inputs["qn_g"], np.float32)
    qn_b = np.asarray(inputs["qn_b"], np.float32)
    kn_g = np.asarray(inputs["kn_g"], np.float32)
    kn_b = np.asarray(inputs["kn_b"], np.float32)
    out_w = np.asarray(inputs["out_w"], np.float32)
    out_b = np.asarray(inputs["out_b"], np.float32)

    import time as _time

    _t = _time.time()
    in_maps = [
        _prep_core_inputs(c, x, Wqkv_w, Wqkv_b, qn_g, qn_b, kn_g, kn_b, out_w)
        for c in range(8)
    ]
    print(f"[kernel] host prep {_time.time()-_t:.1f}s", flush=True)
    _t = _time.time()
    nc = _get_program("brow" in in_maps[0], "tln" in in_maps[0])
    print(f"[kernel] program {_time.time()-_t:.1f}s", flush=True)
    _t = _time.time()
    res = run_bass_kernel_spmd(nc, in_maps, list(range(8)), trace=trace)
    print(f"[kernel] run {_time.time()-_t:.1f}s", flush=True)

    B = x.shape[0]
    bv = Wqkv_b[2 * DIM : 3 * DIM]
    out_bias = out_b + out_w @ bv
    out = np.empty((B, N, DIM), np.float32)
    for b in range(B):
        out[b] = res.results[2 * b]["outp"] + res.results[2 * b + 1]["outp"] + out_bias
    return out, res


def kernel(**inputs):
    out, _ = _run(inputs, trace=False)
    return out
